# revision 7
# baseline (speedup 1.0000x reference)
"""MoE gate (LLaDA2) routing kernel for 8 Trainium2 NeuronCores.

Strategy: token-parallel over 8 cores (2048 tokens/core). Router GEMM as a
single float32r (FP22) matmul per contraction chunk, with x/w pre-rounded to
FP22 on the host so the PE truncation adds no extra error. Grouped top-k
routing on-chip, split across DVE (max8/max_index family) and GPSIMD
(elementwise/reduce ops) so neither engine bottlenecks.
"""
import sys
for p in ("/opt/trn_rl_repo", "/root/.axon_site/_ro/trn_rl_repo"):
    if p not in sys.path:
        sys.path.append(p)

import numpy as np

T, H, E = 16384, 4096, 256
NCORES = 8
TPC = T // NCORES          # tokens per core: 2048
NTILES = TPC // 128        # 16 row tiles
KCH = H // 128             # 32 contraction chunks
WSPLIT = 4                 # w DMA split for early start
G = 8                      # expert groups
GS = E // G                # 32 experts/group
K = 8                      # top-k
NEG = -1.0e4

_cache = {}


def _build():
    import concourse.bacc as bacc
    import concourse.bass as bass
    import concourse.mybir as mybir
    from concourse import tile

    dt = mybir.dt
    Alu = mybir.AluOpType
    Act = mybir.ActivationFunctionType
    Ax = mybir.AxisListType

    nc = bacc.Bacc("TRN2", target_bir_lowering=False, debug=False,
                   num_devices=NCORES)

    x_d = nc.dram_tensor("x", [NTILES, 128, KCH, 128], dt.float32r, kind="ExternalInput")
    w_d = nc.dram_tensor("w", [WSPLIT, 128, KCH // WSPLIT, E], dt.float32r, kind="ExternalInput")
    btab_d = nc.dram_tensor("btab", [128, E], dt.float32, kind="ExternalInput")
    w_out = nc.dram_tensor("w_out", [TPC, K], dt.float32, kind="ExternalOutput")
    i_out = nc.dram_tensor("i_out", [TPC, K], dt.uint32, kind="ExternalOutput")

    KPW = KCH // WSPLIT  # k-chunks per w split

    def bc_mid(ap8, n=8):
        # [128, m] -> [128, n(bcast), m]
        return bass.AP(ap8.tensor, ap8.offset, [list(ap8.ap[0]), [0, n], list(ap8.ap[1])])

    with tile.TileContext(nc) as tc:
        with (
            tc.tile_pool(name="wpool", bufs=1) as wpool,
            tc.tile_pool(name="xpool", bufs=3) as xpool,
            tc.tile_pool(name="ppool", bufs=4, space="PSUM") as ppool,
            tc.tile_pool(name="spool", bufs=2) as spool,
            tc.tile_pool(name="tpool", bufs=2) as tpool,
            tc.tile_pool(name="opool", bufs=1) as opool,
        ):
            wts = []
            for s in range(WSPLIT):
                wt = wpool.tile([128, KPW * E], dt.float32r, tag=f"wt{s}")
                nc.sync.dma_start(wt[:], w_d[s].rearrange("p k e -> p (k e)"))
                wts.append(wt)
            btab = wpool.tile([128, E], dt.float32, tag="btab")
            nc.sync.dma_start(btab[:], btab_d[:])

            out_w = opool.tile([128, NTILES * K], dt.float32, tag="ow")
            out_i = opool.tile([128, NTILES * K], dt.uint32, tag="oi")

            XG = 4                    # x k-group split for DMA granularity
            KPX = KCH // XG
            for i in range(NTILES):
                xgs = []
                for g in range(XG):
                    xg = xpool.tile([128, KPX * 128], dt.float32r, tag=f"xg{g}")
                    nc.sync.dma_start(xg[:], x_d[i, :, g * KPX:(g + 1) * KPX, :]
                                      .rearrange("p k t -> p (k t)"))
                    xgs.append(xg)

                psum = ppool.tile([128, E], dt.float32, tag="ps")
                for k in range(KCH):
                    nc.tensor.matmul(psum[:],
                                     lhsT=xgs[k // KPX][:, (k % KPX) * 128:(k % KPX + 1) * 128],
                                     rhs=wts[k // KPW][:, (k % KPW) * E:(k % KPW + 1) * E],
                                     start=(k == 0), stop=(k == KCH - 1))

                # --- routing epilogue (DVE + GPSIMD split) ---
                scores = spool.tile([128, E], dt.float32, tag="scores")
                nc.scalar.activation(scores[:], psum[:], Act.Sigmoid)

                # sr = scores + bias (selection scores)
                sr = spool.tile([128, E], dt.float32, tag="sr")
                nc.gpsimd.tensor_tensor(sr[:], scores[:], btab[:], Alu.add)
                sr3 = sr[:].rearrange("p (g e) -> p g e", g=G)

                # group top-2: top1 via reduce, knock out top1, reduce again
                top1 = tpool.tile([128, G], dt.float32, tag="top1")
                nc.vector.tensor_reduce(top1[:], sr3, axis=Ax.X, op=Alu.max)
                eqt = spool.tile([128, E], dt.float32, tag="eqt")
                eqt3 = eqt[:].rearrange("p (g e) -> p g e", g=G)
                nc.vector.tensor_tensor(eqt3, sr3, top1[:].to_broadcast([128, G, GS]), Alu.is_equal)
                mrx = spool.tile([128, E], dt.float32, tag="mrx")
                nc.vector.scalar_tensor_tensor(mrx[:], eqt[:], NEG, sr[:],
                                               op0=Alu.mult, op1=Alu.add)
                top2 = tpool.tile([128, G], dt.float32, tag="top2")
                nc.vector.tensor_reduce(top2[:], mrx[:].rearrange("p (g e) -> p g e", g=G),
                                        axis=Ax.X, op=Alu.max)
                gs_t = tpool.tile([128, G], dt.float32, tag="gs")
                nc.gpsimd.tensor_tensor(gs_t[:], top1[:], top2[:], Alu.add)

                # keep top-4 groups: threshold at 4th largest group score
                g8 = tpool.tile([128, 8], dt.float32, tag="g8")
                nc.vector.max(out=g8[:], in_=gs_t[:])
                inv = tpool.tile([128, G], dt.float32, tag="inv")
                nc.vector.tensor_scalar(inv[:], gs_t[:], g8[:, 3:4], -NEG, op0=Alu.is_lt, op1=Alu.mult)
                # mask: sr -= inv (0 for kept groups, 1e4 for dropped)
                nc.gpsimd.tensor_tensor(sr3, sr3, inv[:].to_broadcast([128, G, GS]), Alu.subtract)

                # top-8 selection on masked sr
                vals8 = tpool.tile([128, K], dt.float32, tag="vals8")
                nc.vector.max(out=vals8[:], in_=sr[:])
                idx8 = tpool.tile([128, K], dt.uint32, tag="idx8")
                nc.vector.max_index(out=idx8[:], in_max=vals8[:], in_values=sr[:])

                # scores at selected positions: (sr >= t8) * scores
                selm = spool.tile([128, E], dt.float32, tag="selm")
                nc.vector.scalar_tensor_tensor(selm[:], sr[:], vals8[:, 7:8], scores[:],
                                               op0=Alu.is_ge, op1=Alu.mult)
                svals8 = tpool.tile([128, K], dt.float32, tag="svals8")
                nc.vector.max(out=svals8[:], in_=selm[:])
                sidx8 = tpool.tile([128, K], dt.uint32, tag="sidx8")
                nc.vector.max_index(out=sidx8[:], in_max=svals8[:], in_values=selm[:])

                # reorder svals8 (score-sorted) into idx8 (sr-sorted) slots
                idx8f = tpool.tile([128, K], dt.float32, tag="idx8f")
                nc.gpsimd.tensor_copy(idx8f[:], idx8[:])
                sidx8f = tpool.tile([128, K], dt.float32, tag="sidx8f")
                nc.gpsimd.tensor_copy(sidx8f[:], sidx8[:])
                eq = tpool.tile([128, K * K], dt.float32, tag="eq")
                eq3 = eq[:].rearrange("p (k j) -> p k j", k=K)
                nc.vector.tensor_tensor(eq3, idx8f[:].to_broadcast([128, K, K]), bc_mid(sidx8f[:]), Alu.is_equal)
                prod = tpool.tile([128, K * K], dt.float32, tag="prod")
                prod3 = prod[:].rearrange("p (k j) -> p k j", k=K)
                nc.vector.tensor_tensor(prod3, eq3, bc_mid(svals8[:]), Alu.mult)
                w8 = tpool.tile([128, K], dt.float32, tag="w8")
                nc.vector.tensor_reduce(w8[:], prod3, axis=Ax.X, op=Alu.add)

                sum8 = tpool.tile([128, 1], dt.float32, tag="sum8")
                nc.vector.tensor_reduce(sum8[:], w8[:], axis=Ax.X, op=Alu.add)
                rec = tpool.tile([128, 1], dt.float32, tag="rec")
                nc.vector.reciprocal(rec[:], sum8[:])

                nc.vector.tensor_scalar(out_w[:, i * K:(i + 1) * K], w8[:], rec[:, 0:1], 2.5,
                                        op0=Alu.mult, op1=Alu.mult)
                nc.gpsimd.tensor_copy(out_i[:, i * K:(i + 1) * K], idx8[:])

            nc.sync.dma_start(w_out[:].rearrange("(i p) k -> p i k", p=128),
                              out_w[:].rearrange("p (i k) -> p i k", i=NTILES))
            nc.sync.dma_start(i_out[:].rearrange("(i p) k -> p i k", p=128),
                              out_i[:].rearrange("p (i k) -> p i k", i=NTILES))

    nc.compile()
    return nc


def _rne_fp22(a, m=12):
    # round to m-bit mantissa (HW f32r keeps ~12 bits, truncating)
    drop = 23 - m
    ai = a.view(np.uint32)
    lsb = (ai >> np.uint32(drop)) & np.uint32(1)
    rounded = ai + np.uint32((1 << (drop - 1)) - 1) + lsb
    return (rounded & np.uint32((~((1 << drop) - 1)) & 0xFFFFFFFF)).view(np.float32)


def _prep(hidden_states, weight, expert_bias):
    x = _rne_fp22(np.ascontiguousarray(hidden_states, dtype=np.float32))
    w = _rne_fp22(np.ascontiguousarray(weight, dtype=np.float32))
    # [256, 4096] -> [128p(h), 32k, 256e] -> [WSPLIT, 128, KPW, 256]
    w_l = np.ascontiguousarray(w.reshape(E, KCH, 128).transpose(2, 1, 0))
    w_l = np.ascontiguousarray(w_l.reshape(128, WSPLIT, KCH // WSPLIT, E).transpose(1, 0, 2, 3))
    btab = np.ascontiguousarray(np.broadcast_to(expert_bias.astype(np.float32), (128, E)))

    in_maps = []
    for c in range(NCORES):
        xs = x[c * TPC:(c + 1) * TPC]
        # [2048, 4096] -> [16i, 128p(h), 32k, 128t]
        x_l = np.ascontiguousarray(xs.reshape(NTILES, 128, KCH, 128).transpose(0, 3, 2, 1))
        in_maps.append({"x": x_l, "w": w_l, "btab": btab})
    return in_maps


def kernel(hidden_states, weight, expert_bias, _trace=False):
    from concourse.bass_utils import run_bass_kernel_spmd

    if "nc" not in _cache:
        _cache["nc"] = _build()
    nc = _cache["nc"]
    in_maps = _prep(hidden_states, weight, expert_bias)
    res = run_bass_kernel_spmd(nc, in_maps, core_ids=list(range(NCORES)), trace=_trace)
    _cache["last_results"] = res
    w = np.concatenate([res.results[c]["w_out"] for c in range(NCORES)], axis=0)
    idx = np.concatenate([res.results[c]["i_out"] for c in range(NCORES)], axis=0)
    return w.astype(np.float32), idx.astype(np.int32)
